# revision 45
# baseline (speedup 1.0000x reference)
"""Trainium2 Bass kernel for nn_Adapter (LayerNorm -> 768->64->768 adapter -> residual).

Data parallel over batch: each of the 8 NeuronCores processes one (4096, 768)
slice of x. LN scale/shift and mean-centering are folded into the
down-projection weights on the host:

  pre_relu[t,k] = rstd_t * sum_d w2[k,d]*x[t,d] + beff[k]
    w2[k,d] = w_down[k,d]*ln_w[d] - s[k]/768,  s[k] = sum_d w_down[k,d]*ln_w[d]
    beff[k] = b_down[k] + sum_d w_down[k,d]*ln_b[d]

All on-chip data is bf16 (error ~2e-3 << the 2e-2 gate); the x DMA itself
casts f32->bf16 (SWDGE casting DMA on gpsimd), so the f32 x never lands in
SBUF and the expensive f32 scale pass disappears.

Per group of 4 token tiles (512 tokens) on device:
  DVE bn_stats/bn_aggr -> mean/var; ACT sqrt + DVE recip -> rstd
  DVE xs = x_bf*rstd (bf16 4x mode); PE transposes xs chunks into PSUM;
  DVE/ACT copy PSUM -> xtg [128d, C, 512t]
  PE: 6 accumulating bf16 matmuls -> down PSUM [64, 512]
  ACT relu(down + beff) -> bf16 (ones row for b_up, zero padding rows)
  PE per tile: bf16 up matmuls + bf16 identity matmuls accumulate up + x
  ACT copies PSUM -> SBUF f32, DMA out.
"""
import sys

for _p in ("/opt/trn_rl_repo",):
    if _p not in sys.path:
        sys.path.insert(0, _p)

import numpy as np

import concourse.bacc as bacc
import concourse.mybir as mybir
import concourse.tile as tile
from concourse.bass_utils import run_bass_kernel_spmd

N_CORES = 8
S = 4096          # tokens per core
D = 768           # model dim
K = 64            # bottleneck
P = 128           # partitions / tokens per tile
C = D // P        # 6 d-chunks
NT = S // P       # 32 token tiles per core
GRP = 2           # token tiles per down-matmul group
LN_EPS = 1e-5

F32 = mybir.dt.float32
BF16 = mybir.dt.bfloat16
AF = mybir.ActivationFunctionType
MUL = mybir.AluOpType.mult


def build_nc():
    nc = bacc.Bacc("TRN2", target_bir_lowering=False, debug=False)
    x_d = nc.declare_dram_parameter("x", [S, D], F32, isOutput=False)
    w2t_d = nc.declare_dram_parameter("w2t", [P, C, K], F32, isOutput=False)
    wupt_d = nc.declare_dram_parameter("wupt", [P, D], F32, isOutput=False)
    beff_d = nc.declare_dram_parameter("beff", [K, 1], F32, isOutput=False)
    ident_d = nc.declare_dram_parameter("ident", [P, P], F32, isOutput=False)
    out_d = nc.declare_dram_parameter("out", [S, D], F32, isOutput=True)

    with tile.TileContext(nc) as tc:
        with (
            tc.tile_pool(name="const", bufs=1) as const,
            tc.tile_pool(name="xp", bufs=16) as xpool,
            tc.tile_pool(name="sp", bufs=8) as spool,
            tc.tile_pool(name="dg", bufs=6) as dgp,
            tc.tile_pool(name="xtg", bufs=6) as xtgp,
            tc.tile_pool(name="dt", bufs=6) as dtp,
            tc.tile_pool(name="op", bufs=10) as opool,
            tc.tile_pool(name="ps_t", bufs=3, space="PSUM") as ps_t,
            tc.tile_pool(name="ps_d", bufs=1, space="PSUM") as ps_d,
            tc.tile_pool(name="ps_ua", bufs=3, space="PSUM") as ps_ua,
            tc.tile_pool(name="ps_ub", bufs=1, space="PSUM") as ps_ub,
        ):
            # ---- constants ----
            w2t_f = const.tile([P, C, K], F32)
            nc.sync.dma_start(out=w2t_f, in_=w2t_d.ap())
            w2t_bf = const.tile([P, C, K], BF16)
            nc.vector.tensor_copy(out=w2t_bf, in_=w2t_f)

            wupt_f = const.tile([P, D], F32)
            nc.sync.dma_start(out=wupt_f, in_=wupt_d.ap())
            wupt_bf = const.tile([P, D], BF16)
            nc.vector.tensor_copy(out=wupt_bf, in_=wupt_f)

            beff_sb = const.tile([K, 1], F32)
            nc.sync.dma_start(out=beff_sb, in_=beff_d.ap())

            ident_bf = const.tile([P, P], BF16)
            nc.gpsimd.dma_start(out=ident_bf, in_=ident_d.ap())

            eps_sb = const.tile([P, 1], F32)
            nc.vector.memset(eps_sb, LN_EPS)

            x_ap = x_d.ap()
            out_ap = out_d.ap()

            for g in range(NT // GRP):
                xtg = xtgp.tile([P, C, GRP * P], BF16)   # xs^T, d on partitions
                x_tiles = []
                for j in range(GRP):
                    t = g * GRP + j
                    x_bf = xpool.tile([P, D], BF16)
                    nc.gpsimd.dma_start(out=x_bf, in_=x_ap[t * P:(t + 1) * P, :])
                    stats = spool.tile([P, 3, 6], F32, tag="stats")
                    for si in range(3):
                        nc.vector.bn_stats(
                            out=stats[:, si, :], in_=x_bf[:, si * 256:(si + 1) * 256]
                        )
                    mv = spool.tile([P, 2], F32, tag="mv")
                    nc.vector.bn_aggr(out=mv, in_=stats)
                    std = spool.tile([P, 1], F32, tag="std")
                    nc.scalar.activation(
                        out=std, in_=mv[:, 1:2], func=AF.Sqrt, bias=eps_sb
                    )
                    rstd = spool.tile([P, 1], F32, tag="rstd")
                    nc.vector.reciprocal(out=rstd, in_=std)
                    # diag(rstd): fold the per-token rstd scale into the PE
                    # transpose (out = x_chunk.T @ diag(rstd))
                    diag = dgp.tile([P, P], BF16, tag="diag")
                    nc.scalar.activation(
                        out=diag, in_=ident_bf, func=AF.Copy, scale=rstd
                    )
                    ps_x = ps_t.tile([P, C, P], BF16)
                    for c in range(C):
                        nc.tensor.transpose(
                            out=ps_x[:, c, :],
                            in_=x_bf[:, c * P:(c + 1) * P],
                            identity=diag,
                        )
                    # drain the whole tile's transposes in one copy
                    dst = xtg[:, :, j * P:(j + 1) * P]
                    if j % 2 == 1:
                        nc.vector.tensor_copy(out=dst, in_=ps_x)
                    else:
                        nc.scalar.copy(out=dst, in_=ps_x)
                    x_tiles.append(x_bf)

                # ---- down projection for the whole group: PSUM [64, 512] ----
                ps_dt = ps_d.tile([K, GRP * P], F32)
                for c in range(C):
                    nc.tensor.matmul(
                        out=ps_dt, lhsT=w2t_bf[:, c, :], rhs=xtg[:, c, :],
                        start=(c == 0), stop=(c == C - 1),
                    )
                dt = dtp.tile([P, GRP * P], BF16)
                nc.gpsimd.memset(dt[K:P, :], 0.0)          # padding rows
                nc.gpsimd.memset(dt[K:K + 1, :], 1.0)      # ones row -> b_up
                nc.scalar.activation(
                    out=dt[0:K, :], in_=ps_dt, func=AF.Relu, bias=beff_sb, scale=1.0
                )

                # ---- up projection + residual, per tile ----
                for j in range(GRP):
                    t = g * GRP + j
                    lhs_j = dt[:, j * P:(j + 1) * P]
                    pa = ps_ua.tile([P, 512], F32)
                    pb = ps_ub.tile([P, 256], F32)
                    nc.tensor.matmul(out=pa, lhsT=lhs_j,
                                     rhs=wupt_bf[:, 0:512], start=True, stop=False)
                    nc.tensor.matmul(out=pb, lhsT=lhs_j,
                                     rhs=wupt_bf[:, 512:768], start=True, stop=False)
                    x_r = x_tiles[j]
                    nc.tensor.matmul(out=pa, lhsT=ident_bf,
                                     rhs=x_r[:, 0:512], start=False, stop=True)
                    nc.tensor.matmul(out=pb, lhsT=ident_bf,
                                     rhs=x_r[:, 512:768], start=False, stop=True)
                    o = opool.tile([P, D], F32)
                    nc.scalar.copy(out=o[:, 0:512], in_=pa)
                    nc.vector.tensor_copy(out=o[:, 512:768], in_=pb)
                    nc.sync.dma_start(out=out_ap[t * P:(t + 1) * P, :], in_=o)

    nc.compile()
    return nc


def host_weights(ln_w, ln_b, w_down, b_down, w_up, b_up):
    ln_w = ln_w.astype(np.float64)
    ln_b = ln_b.astype(np.float64)
    w_down = w_down.astype(np.float64)
    w_up = w_up.astype(np.float64)
    w2 = w_down * ln_w[None, :]                      # [K, D]
    s = w2.sum(axis=1)                               # [K]
    w2c = w2 - s[:, None] / D
    beff = b_down.astype(np.float64) + w_down @ ln_b  # [K]
    w2t = np.ascontiguousarray(
        w2c.T.reshape(C, P, K).transpose(1, 0, 2)
    ).astype(np.float32)                             # [P, C, K]
    wupt = np.zeros((P, D), np.float32)
    wupt[:K] = w_up.T.astype(np.float32)
    wupt[K] = b_up.astype(np.float32)
    return {
        "w2t": w2t,
        "wupt": wupt,
        "beff": beff.astype(np.float32).reshape(K, 1),
        "ident": np.eye(P, dtype=np.float32),
    }


_NC = None


def _get_nc():
    global _NC
    if _NC is None:
        _NC = build_nc()
    return _NC


def run_spmd(in_maps, trace=False, **kw):
    return run_bass_kernel_spmd(
        _get_nc(), in_maps, core_ids=list(range(N_CORES)), trace=trace, **kw
    )


def kernel(x, ln_w, ln_b, w_down, b_down, w_up, b_up):
    x = np.asarray(x, dtype=np.float32)
    w = host_weights(
        np.asarray(ln_w), np.asarray(ln_b), np.asarray(w_down),
        np.asarray(b_down), np.asarray(w_up), np.asarray(b_up),
    )
    in_maps = [{"x": np.ascontiguousarray(x[c]), **w} for c in range(N_CORES)]
    res = run_spmd(in_maps)
    return np.stack([res.results[c]["out"] for c in range(N_CORES)], axis=0)


# revision 46
# speedup vs baseline: 1.0749x; 1.0749x over previous
"""Trainium2 Bass kernel for nn_Adapter (LayerNorm -> 768->64->768 adapter -> residual).

Data parallel over batch: each of the 8 NeuronCores processes one (4096, 768)
slice of x. LN scale/shift and mean-centering are folded into the
down-projection weights on the host:

  pre_relu[t,k] = rstd_t * sum_d w2[k,d]*x[t,d] + beff[k]
    w2[k,d] = w_down[k,d]*ln_w[d] - s[k]/768,  s[k] = sum_d w_down[k,d]*ln_w[d]
    beff[k] = b_down[k] + sum_d w_down[k,d]*ln_b[d]

All on-chip data is bf16 (error ~2e-3 << the 2e-2 gate); the x DMA itself
casts f32->bf16 (SWDGE casting DMA on gpsimd), so the f32 x never lands in
SBUF and the expensive f32 scale pass disappears.

Per group of 4 token tiles (512 tokens) on device:
  DVE bn_stats/bn_aggr -> mean/var; ACT sqrt + DVE recip -> rstd
  DVE xs = x_bf*rstd (bf16 4x mode); PE transposes xs chunks into PSUM;
  DVE/ACT copy PSUM -> xtg [128d, C, 512t]
  PE: 6 accumulating bf16 matmuls -> down PSUM [64, 512]
  ACT relu(down + beff) -> bf16 (ones row for b_up, zero padding rows)
  PE per tile: bf16 up matmuls + bf16 identity matmuls accumulate up + x
  ACT copies PSUM -> SBUF f32, DMA out.
"""
import sys

for _p in ("/opt/trn_rl_repo",):
    if _p not in sys.path:
        sys.path.insert(0, _p)

import numpy as np

import concourse.bacc as bacc
import concourse.mybir as mybir
import concourse.tile as tile
from concourse.bass_utils import run_bass_kernel_spmd

N_CORES = 8
S = 4096          # tokens per core
D = 768           # model dim
K = 64            # bottleneck
P = 128           # partitions / tokens per tile
C = D // P        # 6 d-chunks
NT = S // P       # 32 token tiles per core
GRP = 2           # token tiles per down-matmul group
LN_EPS = 1e-5

F32 = mybir.dt.float32
BF16 = mybir.dt.bfloat16
AF = mybir.ActivationFunctionType
MUL = mybir.AluOpType.mult


def build_nc():
    nc = bacc.Bacc("TRN2", target_bir_lowering=False, debug=False)
    x_d = nc.declare_dram_parameter("x", [S, D], F32, isOutput=False)
    w2t_d = nc.declare_dram_parameter("w2t", [P, C, K], F32, isOutput=False)
    wupt_d = nc.declare_dram_parameter("wupt", [P, D], F32, isOutput=False)
    beff_d = nc.declare_dram_parameter("beff", [K, 1], F32, isOutput=False)
    ident_d = nc.declare_dram_parameter("ident", [P, P], F32, isOutput=False)
    out_d = nc.declare_dram_parameter("out", [S, D], F32, isOutput=True)

    with tile.TileContext(nc) as tc:
        with (
            tc.tile_pool(name="const", bufs=1) as const,
            tc.tile_pool(name="xp", bufs=16) as xpool,
            tc.tile_pool(name="sp", bufs=8) as spool,
            tc.tile_pool(name="dg", bufs=6) as dgp,
            tc.tile_pool(name="xtg", bufs=6) as xtgp,
            tc.tile_pool(name="dt", bufs=6) as dtp,
            tc.tile_pool(name="op", bufs=10) as opool,
            tc.tile_pool(name="ps_t", bufs=3, space="PSUM") as ps_t,
            tc.tile_pool(name="ps_d", bufs=1, space="PSUM") as ps_d,
            tc.tile_pool(name="ps_ua", bufs=3, space="PSUM") as ps_ua,
            tc.tile_pool(name="ps_ub", bufs=1, space="PSUM") as ps_ub,
        ):
            # ---- constants ----
            w2t_f = const.tile([P, C, K], F32)
            nc.sync.dma_start(out=w2t_f, in_=w2t_d.ap())
            w2t_bf = const.tile([P, C, K], BF16)
            nc.vector.tensor_copy(out=w2t_bf, in_=w2t_f)

            wupt_f = const.tile([P, D], F32)
            nc.sync.dma_start(out=wupt_f, in_=wupt_d.ap())
            wupt_bf = const.tile([P, D], BF16)
            nc.vector.tensor_copy(out=wupt_bf, in_=wupt_f)

            beff_sb = const.tile([K, 1], F32)
            nc.sync.dma_start(out=beff_sb, in_=beff_d.ap())

            ident_bf = const.tile([P, P], BF16)
            nc.gpsimd.dma_start(out=ident_bf, in_=ident_d.ap())

            eps_sb = const.tile([P, 1], F32)
            nc.vector.memset(eps_sb, LN_EPS)

            x_ap = x_d.ap()
            out_ap = out_d.ap()

            for g in range(NT // GRP):
                xtg = xtgp.tile([P, C, GRP * P], BF16)   # xs^T, d on partitions
                x_tiles = []
                for j in range(GRP):
                    t = g * GRP + j
                    x_bf = xpool.tile([P, D], BF16)
                    nc.gpsimd.dma_start(out=x_bf, in_=x_ap[t * P:(t + 1) * P, :])
                    stats = spool.tile([P, 3, 6], F32, tag="stats")
                    for si in range(3):
                        nc.vector.bn_stats(
                            out=stats[:, si, :], in_=x_bf[:, si * 256:(si + 1) * 256]
                        )
                    mv = spool.tile([P, 2], F32, tag="mv")
                    nc.vector.bn_aggr(out=mv, in_=stats)
                    std = spool.tile([P, 1], F32, tag="std")
                    nc.scalar.activation(
                        out=std, in_=mv[:, 1:2], func=AF.Sqrt, bias=eps_sb
                    )
                    rstd = spool.tile([P, 1], F32, tag="rstd")
                    nc.vector.reciprocal(out=rstd, in_=std)
                    # diag(rstd): fold the per-token rstd scale into the PE
                    # transpose (out = x_chunk.T @ diag(rstd))
                    diag = dgp.tile([P, P], BF16, tag="diag")
                    nc.vector.tensor_scalar(
                        out=diag, in0=ident_bf, scalar1=rstd, scalar2=None, op0=MUL,
                    )
                    ps_x = ps_t.tile([P, C, P], BF16)
                    for c in range(C):
                        nc.tensor.transpose(
                            out=ps_x[:, c, :],
                            in_=x_bf[:, c * P:(c + 1) * P],
                            identity=diag,
                        )
                    # drain the whole tile's transposes in one copy
                    dst = xtg[:, :, j * P:(j + 1) * P]
                    if j % 2 == 1:
                        nc.vector.tensor_copy(out=dst, in_=ps_x)
                    else:
                        nc.scalar.copy(out=dst, in_=ps_x)
                    x_tiles.append(x_bf)

                # ---- down projection for the whole group: PSUM [64, 512] ----
                ps_dt = ps_d.tile([K, GRP * P], F32)
                for c in range(C):
                    nc.tensor.matmul(
                        out=ps_dt, lhsT=w2t_bf[:, c, :], rhs=xtg[:, c, :],
                        start=(c == 0), stop=(c == C - 1),
                    )
                dt = dtp.tile([P, GRP * P], BF16)
                nc.gpsimd.memset(dt[K:P, :], 0.0)          # padding rows
                nc.gpsimd.memset(dt[K:K + 1, :], 1.0)      # ones row -> b_up
                nc.scalar.activation(
                    out=dt[0:K, :], in_=ps_dt, func=AF.Relu, bias=beff_sb, scale=1.0
                )

                # ---- up projection + residual, per tile ----
                for j in range(GRP):
                    t = g * GRP + j
                    lhs_j = dt[:, j * P:(j + 1) * P]
                    pa = ps_ua.tile([P, 512], F32)
                    pb = ps_ub.tile([P, 256], F32)
                    nc.tensor.matmul(out=pa, lhsT=lhs_j,
                                     rhs=wupt_bf[:, 0:512], start=True, stop=False)
                    nc.tensor.matmul(out=pb, lhsT=lhs_j,
                                     rhs=wupt_bf[:, 512:768], start=True, stop=False)
                    x_r = x_tiles[j]
                    nc.tensor.matmul(out=pa, lhsT=ident_bf,
                                     rhs=x_r[:, 0:512], start=False, stop=True)
                    nc.tensor.matmul(out=pb, lhsT=ident_bf,
                                     rhs=x_r[:, 512:768], start=False, stop=True)
                    o = opool.tile([P, D], F32)
                    nc.scalar.copy(out=o[:, 0:512], in_=pa)
                    nc.vector.tensor_copy(out=o[:, 512:768], in_=pb)
                    nc.sync.dma_start(out=out_ap[t * P:(t + 1) * P, :], in_=o)

    nc.compile()
    return nc


def host_weights(ln_w, ln_b, w_down, b_down, w_up, b_up):
    ln_w = ln_w.astype(np.float64)
    ln_b = ln_b.astype(np.float64)
    w_down = w_down.astype(np.float64)
    w_up = w_up.astype(np.float64)
    w2 = w_down * ln_w[None, :]                      # [K, D]
    s = w2.sum(axis=1)                               # [K]
    w2c = w2 - s[:, None] / D
    beff = b_down.astype(np.float64) + w_down @ ln_b  # [K]
    w2t = np.ascontiguousarray(
        w2c.T.reshape(C, P, K).transpose(1, 0, 2)
    ).astype(np.float32)                             # [P, C, K]
    wupt = np.zeros((P, D), np.float32)
    wupt[:K] = w_up.T.astype(np.float32)
    wupt[K] = b_up.astype(np.float32)
    return {
        "w2t": w2t,
        "wupt": wupt,
        "beff": beff.astype(np.float32).reshape(K, 1),
        "ident": np.eye(P, dtype=np.float32),
    }


_NC = None


def _get_nc():
    global _NC
    if _NC is None:
        _NC = build_nc()
    return _NC


def run_spmd(in_maps, trace=False, **kw):
    return run_bass_kernel_spmd(
        _get_nc(), in_maps, core_ids=list(range(N_CORES)), trace=trace, **kw
    )


def kernel(x, ln_w, ln_b, w_down, b_down, w_up, b_up):
    x = np.asarray(x, dtype=np.float32)
    w = host_weights(
        np.asarray(ln_w), np.asarray(ln_b), np.asarray(w_down),
        np.asarray(b_down), np.asarray(w_up), np.asarray(b_up),
    )
    in_maps = [{"x": np.ascontiguousarray(x[c]), **w} for c in range(N_CORES)]
    res = run_spmd(in_maps)
    return np.stack([res.results[c]["out"] for c in range(N_CORES)], axis=0)
